# revision 3
# baseline (speedup 1.0000x reference)
"""MoE (top-2 of 16 experts, SwiGLU MLP) kernel for 8 Trainium2 NeuronCores.

Strategy (expert-parallel, per sharding hint):
  - Host: router (x @ w_gate -> softmax -> top-2) computed in float64,
    tokens gathered per expert ("all-to-all"). Experts ranked by token
    count: the 8 largest go in core slot 0, the 8 smallest in slot 1;
    each slot's capacity is the exact max count in that slot (rounded to
    a multiple of 4), so padding waste is ~0.2%.
  - Device (SPMD over 8 cores, 2 experts/core), all operands bf16 with
    fp32 PSUM accumulation (halves HBM traffic vs fp32; rel err ~4e-3):
        ht = silu(W1e.T @ Xt) * (W2e.T @ Xt)     [feature-major layout]
        yt = (WCe.T @ ht) * gate_row
  - Host: scatter-add per-expert outputs back to token order (fp32).
"""

import contextlib
import ctypes
import os
import sys
import types

sys.path.insert(0, "/opt/trn_rl_repo")

import numpy as np
import ml_dtypes

import concourse.bass as bass
import concourse.mybir as mybir
import concourse.tile as tile

BF16 = ml_dtypes.bfloat16
EMB = 1024
HID = 1024
E = 16
TOPK = 2
NCORES = 8
EPC = E // NCORES  # experts per core
P = 128
KT = EMB // P  # contraction tiles per GEMM (8)
HT = HID // P  # hidden/output row-blocks (8)


def _install_profile_shim():
    """Register the axon NTFF profiling hook (missing antenv.axon_hooks in
    this image) so run_bass_kernel_spmd(trace=True) can measure HW time."""
    if "antenv.axon_hooks" in sys.modules:
        return
    try:
        lib = ctypes.CDLL("/opt/axon/libaxon_pjrt.so")
        lib.axon_start_nrt_profile.argtypes = [
            ctypes.POINTER(ctypes.c_int64),
            ctypes.c_size_t,
        ]
        lib.axon_start_nrt_profile.restype = ctypes.c_int64
        lib.axon_stop_nrt_profile.argtypes = [ctypes.c_char_p]
        lib.axon_stop_nrt_profile.restype = ctypes.c_int64
    except Exception:
        return

    @contextlib.contextmanager
    def _hook(output_dir, device_ids):
        import jax

        jax.devices()
        ids = (
            (ctypes.c_int64 * len(device_ids))(*device_ids) if device_ids else None
        )
        rc = lib.axon_start_nrt_profile(ids, len(device_ids) if device_ids else 0)
        if rc != 0:
            raise RuntimeError(f"axon_start_nrt_profile rc={rc}")
        try:
            yield
        finally:
            n = lib.axon_stop_nrt_profile(str(output_dir).encode())
            print(f"profile: {n} file(s) written to {output_dir}")

    mod = types.ModuleType("antenv.axon_hooks")
    mod.get_axon_ntff_profile_hook = lambda: _hook
    mod.set_axon_ntff_profile_hook = lambda h: None
    sys.modules["antenv.axon_hooks"] = mod


def _split_multi_waits(nc):
    """This container's walrus only encodes one sem wait per CTRL-class
    instruction; hoist extra waits onto dedicated single-wait NoOps."""
    idx = 0
    for fn in nc.m.functions:
        for bb in fn.blocks:
            new = []
            for inst in bb.instructions:
                si = inst.sync_info
                if si is not None and len(si.on_wait) > 1:
                    waits = list(si.on_wait)
                    for w in waits[:-1]:
                        c = mybir.InstNoOp(name=f"wsplit-{idx}", ins=[], outs=[])
                        idx += 1
                        c.engine = inst.engine
                        c.sync_info = mybir.SyncInfo(on_wait=[w], on_update=[])
                        new.append(c)
                    si.on_wait = [waits[-1]]
                new.append(inst)
            bb.instructions = new


def _token_chunks(C):
    """Split C (multiple of 4) into near-equal chunks <=512, multiples of 4."""
    n = max(1, -(-C // 512))
    base = C // n
    base -= base % 4
    sizes = [base] * n
    rem = C - base * n
    i = 0
    while rem > 0:
        add = min(4, rem)
        sizes[i] += add
        rem -= add
        i = (i + 1) % n
    return [s for s in sizes if s > 0]


def _build_bass(caps):
    F32 = mybir.dt.float32
    B16 = mybir.dt.bfloat16

    nc = bass.Bass()
    xt_d, g_d, w1_d, w2_d, wc_d, yt_d = [], [], [], [], [], []
    for e in range(EPC):
        C = caps[e]
        # xt: [p, k, c] layout flattened to [P, KT*C]
        xt_d.append(nc.declare_dram_parameter(f"xt{e}", [P, KT * C], B16, isOutput=False))
        g_d.append(nc.declare_dram_parameter(f"g{e}", [P, C], F32, isOutput=False))
        # w1/w2: [p, h, k, q] layout -> [P, HT*KT*P]; per-h DMA is contiguous
        w1_d.append(
            nc.declare_dram_parameter(f"w1_{e}", [P, HT * KT * P], B16, isOutput=False)
        )
        w2_d.append(
            nc.declare_dram_parameter(f"w2_{e}", [P, HT * KT * P], B16, isOutput=False)
        )
        # wc: [p, d, h, q] layout -> [P, HT*HT*P]
        wc_d.append(
            nc.declare_dram_parameter(f"wc_{e}", [P, HT * HT * P], B16, isOutput=False)
        )
        # yt: [p, d, c] layout -> [P, HT*C]
        yt_d.append(nc.declare_dram_parameter(f"yt{e}", [P, HT * C], B16, isOutput=True))

    with tile.TileContext(nc) as tc:
        with (
            tc.tile_pool(name="xt", bufs=2) as xt_pool,
            tc.tile_pool(name="ht", bufs=2) as ht_pool,
            tc.tile_pool(name="g", bufs=2) as g_pool,
            tc.tile_pool(name="w12", bufs=4) as w12_pool,
            tc.tile_pool(name="wc", bufs=4) as wc_pool,
            tc.tile_pool(name="s", bufs=3) as s_pool,
            tc.tile_pool(name="y", bufs=3) as y_pool,
            tc.tile_pool(name="psA", bufs=2, space="PSUM") as psA,
            tc.tile_pool(name="psB", bufs=3, space="PSUM") as psB,
            tc.tile_pool(name="psW", bufs=1, space="PSUM") as psW,
        ):
            # PE p-state warm-up: dependency-free matmuls on a memset tile
            # run during the initial DMA wait so the clock ramps before the
            # first real matmul.
            warm_in = s_pool.tile([P, 512], mybir.dt.bfloat16, tag="warm")
            nc.any.memset(warm_in[:], 0.0)
            warm_ps = psW.tile([P, 512], mybir.dt.float32, tag="warmps")
            for _ in range(8):
                nc.tensor.matmul(
                    warm_ps[:], warm_in[:, 0:P], warm_in[:], start=True, stop=True
                )

            for e in range(EPC):
                C = caps[e]
                chunks = _token_chunks(C)
                # h=0 weights first: they + the first xt chunk gate the
                # first matmul, so they must not queue behind bulk input DMA.
                w1t0 = w12_pool.tile([P, KT, P], B16, tag="w1")
                w2t0 = w12_pool.tile([P, KT, P], B16, tag="w2")
                nc.sync.dma_start(w1t0[:], w1_d[e][:, 0 : KT * P])
                xt_sb = xt_pool.tile([P, KT, C], B16, tag="xt")
                xt_view = xt_d[e].rearrange("p (k c) -> p k c", k=KT)
                c0 = 0
                for cs in chunks:
                    for k in range(KT):
                        nc.sync.dma_start(
                            xt_sb[:, k, c0 : c0 + cs],
                            xt_view[:, k, c0 : c0 + cs],
                        )
                    if c0 == 0:
                        nc.sync.dma_start(w2t0[:], w2_d[e][:, 0 : KT * P])
                    c0 += cs
                ht_sb = ht_pool.tile([P, HT, C], B16, tag="ht")

                # Phase A: ht = silu(W1.T @ Xt) * (W2.T @ Xt)
                for h in range(HT):
                    if h == 0:
                        w1t, w2t = w1t0, w2t0
                    else:
                        w1t = w12_pool.tile([P, KT, P], B16, tag="w1")
                        w2t = w12_pool.tile([P, KT, P], B16, tag="w2")
                        nc.sync.dma_start(
                            w1t[:], w1_d[e][:, h * KT * P : (h + 1) * KT * P]
                        )
                        nc.sync.dma_start(
                            w2t[:], w2_d[e][:, h * KT * P : (h + 1) * KT * P]
                        )
                    c0 = 0
                    for cs in chunks:
                        ps1 = psA.tile([P, cs], mybir.dt.float32, tag="ps1")
                        ps2 = psA.tile([P, cs], mybir.dt.float32, tag="ps2")
                        for k in range(KT):
                            nc.tensor.matmul(
                                ps1[:],
                                w1t[:, k, :],
                                xt_sb[:, k, c0 : c0 + cs],
                                start=(k == 0),
                                stop=(k == KT - 1),
                            )
                        for k in range(KT):
                            nc.tensor.matmul(
                                ps2[:],
                                w2t[:, k, :],
                                xt_sb[:, k, c0 : c0 + cs],
                                start=(k == 0),
                                stop=(k == KT - 1),
                            )
                        s_sb = s_pool.tile([P, 512], mybir.dt.float32, tag="s")
                        nc.scalar.activation(
                            s_sb[:, :cs],
                            ps1[:],
                            mybir.ActivationFunctionType.Silu,
                        )
                        nc.vector.tensor_mul(
                            ht_sb[:, h, c0 : c0 + cs], s_sb[:, :cs], ps2[:]
                        )
                        c0 += cs

                # Phase B: yt = (WC.T @ ht) * gate
                g_sb = g_pool.tile([P, C], F32, tag="g")
                nc.sync.dma_start(g_sb[:], g_d[e][:])
                yt_view = yt_d[e].rearrange("p (d c) -> p d c", d=HT)
                for d in range(HT):
                    wct = wc_pool.tile([P, HT, P], B16, tag="wc")
                    nc.sync.dma_start(
                        wct[:], wc_d[e][:, d * HT * P : (d + 1) * HT * P]
                    )
                    c0 = 0
                    for cs in chunks:
                        psy = psB.tile([P, cs], mybir.dt.float32, tag="psy")
                        for h in range(HT):
                            nc.tensor.matmul(
                                psy[:],
                                wct[:, h, :],
                                ht_sb[:, h, c0 : c0 + cs],
                                start=(h == 0),
                                stop=(h == HT - 1),
                            )
                        y_sb = y_pool.tile([P, 512], B16, tag="y")
                        nc.vector.tensor_mul(
                            y_sb[:, :cs], psy[:], g_sb[:, c0 : c0 + cs]
                        )
                        nc.sync.dma_start(
                            yt_view[:, d, c0 : c0 + cs],
                            y_sb[:, :cs],
                        )
                        c0 += cs

    _split_multi_waits(nc)
    return nc


def _pack_w12(w):
    """[EMB, HID] -> [P, HT*KT*P] with layout [p, h, k, q]."""
    t = np.ascontiguousarray(
        w.astype(BF16).reshape(KT, P, HT, P).transpose(1, 2, 0, 3)
    )
    return t.reshape(P, HT * KT * P)


def _pack_wc(w):
    """[HID, EMB] -> [P, HT*HT*P] with layout [p, d, h, q]."""
    t = np.ascontiguousarray(
        w.astype(BF16).reshape(HT, P, HT, P).transpose(1, 2, 0, 3)
    )
    return t.reshape(P, HT * HT * P)


def kernel(x, w_gate, w1, w2, wc):
    trace = bool(int(os.environ.get("BASS_MOE_TRACE", "0")))
    if trace:
        _install_profile_shim()

    import concourse.bass_utils as bass_utils

    bass_utils.upload_artifacts = lambda tmpdir: f"local://{tmpdir}"

    x = np.asarray(x, dtype=np.float32)
    w_gate = np.asarray(w_gate, dtype=np.float32)
    w1 = np.asarray(w1, dtype=np.float32)
    w2 = np.asarray(w2, dtype=np.float32)
    wc = np.asarray(wc, dtype=np.float32)

    b, s, d = x.shape
    xf = x.reshape(-1, d)
    n = xf.shape[0]

    # ---- Router on host (float64: stable ranking + gate values) ----
    logits = xf.astype(np.float64) @ w_gate.astype(np.float64)
    mx = logits.max(axis=1, keepdims=True)
    p = np.exp(logits - mx)
    p /= p.sum(axis=1, keepdims=True)
    top = np.argpartition(-logits, TOPK, axis=1)[:, :TOPK]  # top-2 ids (unordered)

    sel_tok = []  # per expert: token indices
    sel_gate = []  # per expert: gate values
    flat_e = top.ravel()
    flat_t = np.repeat(np.arange(n), TOPK)
    order = np.argsort(flat_e, kind="stable")
    se, st = flat_e[order], flat_t[order]
    bounds = np.searchsorted(se, np.arange(E + 1))
    counts = np.diff(bounds)
    for e in range(E):
        toks = st[bounds[e] : bounds[e + 1]]
        sel_tok.append(toks)
        sel_gate.append(p[toks, e].astype(np.float32))

    # ---- Slot assignment: biggest experts in slot 0, smallest in slot 1,
    # so each slot's uniform capacity hugs its experts' actual counts ----
    rank = np.argsort(-counts, kind="stable")
    slot_experts = [
        [int(rank[core + j * NCORES]) for j in range(EPC)] for core in range(NCORES)
    ]
    caps = []
    for j in range(EPC):
        cmax = max(counts[slot_experts[core][j]] for core in range(NCORES))
        caps.append(max(4, int(-(-cmax // 4) * 4)))

    # ---- Build per-core input maps (expert-parallel: 2 experts/core) ----
    xf_b = xf.astype(BF16)
    in_maps = []
    for core in range(NCORES):
        m = {}
        for j in range(EPC):
            e = slot_experts[core][j]
            C = caps[j]
            toks = sel_tok[e]
            # xt layout [p, k, c]
            xt = np.zeros((P, KT, C), dtype=BF16)
            xt[:, :, : len(toks)] = (
                xf_b[toks].reshape(len(toks), KT, P).transpose(2, 1, 0)
            )
            g = np.zeros((C,), dtype=np.float32)
            g[: len(toks)] = sel_gate[e]
            m[f"xt{j}"] = xt.reshape(P, KT * C)
            m[f"g{j}"] = np.broadcast_to(g, (P, C)).copy()
            m[f"w1_{j}"] = _pack_w12(w1[e])
            m[f"w2_{j}"] = _pack_w12(w2[e])
            m[f"wc_{j}"] = _pack_wc(wc[e])
        in_maps.append(m)

    nc = _build_bass(caps)
    res = bass_utils.run_bass_kernel_spmd(
        nc, in_maps, list(range(NCORES)), trace=trace
    )
    if trace:
        kernel.last_exec_time_ns = res.exec_time_ns
        kernel.last_trace = (
            res.instructions_and_trace[1] if res.instructions_and_trace else None
        )

    # ---- Scatter-add back to token order ----
    out = np.zeros((n, d), dtype=np.float32)
    for core in range(NCORES):
        for j in range(EPC):
            e = slot_experts[core][j]
            toks = sel_tok[e]
            C = caps[j]
            yt = (
                np.asarray(res.results[core][f"yt{j}"])
                .reshape(P, HT, C)
                .transpose(1, 0, 2)
                .reshape(EMB, C)
                .astype(np.float32)
            )
            out[toks] += yt[:, : len(toks)].T
    return out.reshape(b, s, d)


# revision 5
# speedup vs baseline: 1.0235x; 1.0235x over previous
"""MoE (top-2 of 16 experts, SwiGLU MLP) kernel for 8 Trainium2 NeuronCores.

Strategy (expert-parallel, per sharding hint):
  - Host: router (x @ w_gate -> softmax -> top-2) computed in float64,
    tokens gathered per expert ("all-to-all"). Experts ranked by token
    count: the 8 largest go in core slot 0, the 8 smallest in slot 1;
    each slot's capacity is the exact max count in that slot (rounded to
    a multiple of 4), so padding waste is ~0.2%.
  - Device (SPMD over 8 cores, 2 experts/core), all operands bf16 with
    fp32 PSUM accumulation (halves HBM traffic vs fp32; rel err ~4e-3):
        ht = silu(W1e.T @ Xt) * (W2e.T @ Xt)     [feature-major layout]
        yt = (WCe.T @ ht) * gate_row
    Phase A runs chunk-outer/h-inner so the first token chunk (one DMA)
    unlocks ~20us of PE work -- the PE never waits for the bulk of xt.
  - Host: scatter-add per-expert outputs back to token order (fp32).
"""

import contextlib
import ctypes
import os
import sys
import types

sys.path.insert(0, "/opt/trn_rl_repo")

import numpy as np
import ml_dtypes

import concourse.bass as bass
import concourse.mybir as mybir
import concourse.tile as tile

BF16 = ml_dtypes.bfloat16
EMB = 1024
HID = 1024
E = 16
TOPK = 2
NCORES = 8
EPC = E // NCORES  # experts per core
P = 128
KT = EMB // P  # contraction tiles per GEMM (8)
HT = HID // P  # hidden/output row-blocks (8)


def _install_profile_shim():
    """Register the axon NTFF profiling hook (missing antenv.axon_hooks in
    this image) so run_bass_kernel_spmd(trace=True) can measure HW time."""
    if "antenv.axon_hooks" in sys.modules:
        return
    try:
        lib = ctypes.CDLL("/opt/axon/libaxon_pjrt.so")
        lib.axon_start_nrt_profile.argtypes = [
            ctypes.POINTER(ctypes.c_int64),
            ctypes.c_size_t,
        ]
        lib.axon_start_nrt_profile.restype = ctypes.c_int64
        lib.axon_stop_nrt_profile.argtypes = [ctypes.c_char_p]
        lib.axon_stop_nrt_profile.restype = ctypes.c_int64
    except Exception:
        return

    @contextlib.contextmanager
    def _hook(output_dir, device_ids):
        import jax

        jax.devices()
        ids = (
            (ctypes.c_int64 * len(device_ids))(*device_ids) if device_ids else None
        )
        rc = lib.axon_start_nrt_profile(ids, len(device_ids) if device_ids else 0)
        if rc != 0:
            raise RuntimeError(f"axon_start_nrt_profile rc={rc}")
        try:
            yield
        finally:
            n = lib.axon_stop_nrt_profile(str(output_dir).encode())
            print(f"profile: {n} file(s) written to {output_dir}")

    mod = types.ModuleType("antenv.axon_hooks")
    mod.get_axon_ntff_profile_hook = lambda: _hook
    mod.set_axon_ntff_profile_hook = lambda h: None
    sys.modules["antenv.axon_hooks"] = mod


def _split_multi_waits(nc):
    """This container's walrus only encodes one sem wait per CTRL-class
    instruction; hoist extra waits onto dedicated single-wait NoOps."""
    idx = 0
    for fn in nc.m.functions:
        for bb in fn.blocks:
            new = []
            for inst in bb.instructions:
                si = inst.sync_info
                if si is not None and len(si.on_wait) > 1:
                    waits = list(si.on_wait)
                    for w in waits[:-1]:
                        c = mybir.InstNoOp(name=f"wsplit-{idx}", ins=[], outs=[])
                        idx += 1
                        c.engine = inst.engine
                        c.sync_info = mybir.SyncInfo(on_wait=[w], on_update=[])
                        new.append(c)
                    si.on_wait = [waits[-1]]
                new.append(inst)
            bb.instructions = new


def _token_chunks(C):
    """Split C (multiple of 4) into near-equal chunks <=512, multiples of 4."""
    n = max(1, -(-C // 512))
    base = C // n
    base -= base % 4
    sizes = [base] * n
    rem = C - base * n
    i = 0
    while rem > 0:
        add = min(4, rem)
        sizes[i] += add
        rem -= add
        i = (i + 1) % n
    return [s for s in sizes if s > 0]


def _build_bass(caps):
    F32 = mybir.dt.float32
    B16 = mybir.dt.bfloat16

    nc = bass.Bass()
    xt_d, g_d, w12_d, wc_d, yt_d = [], [], [], [], []
    for e in range(EPC):
        C = caps[e]
        # xt: per-chunk blocks of [p, k, cs] -> flat [P, KT*C]; one DMA/chunk
        xt_d.append(nc.declare_dram_parameter(f"xt{e}", [P, KT * C], B16, isOutput=False))
        g_d.append(nc.declare_dram_parameter(f"g{e}", [P, C], F32, isOutput=False))
        # w1+w2 combined: [p, h, two, k, q] -> [P, HT*2*KT*P]; one DMA per h
        w12_d.append(
            nc.declare_dram_parameter(
                f"w12_{e}", [P, HT * 2 * KT * P], B16, isOutput=False
            )
        )
        # wc: [p, d, h, q] layout -> [P, HT*HT*P]
        wc_d.append(
            nc.declare_dram_parameter(f"wc_{e}", [P, HT * HT * P], B16, isOutput=False)
        )
        # yt: [p, d, c] layout -> [P, HT*C]
        yt_d.append(nc.declare_dram_parameter(f"yt{e}", [P, HT * C], B16, isOutput=True))

    with tile.TileContext(nc) as tc:
        with (
            tc.tile_pool(name="xt", bufs=3) as xt_pool,
            tc.tile_pool(name="ht", bufs=2) as ht_pool,
            tc.tile_pool(name="g", bufs=2) as g_pool,
            tc.tile_pool(name="w12", bufs=9) as w12_pool,
            tc.tile_pool(name="wc", bufs=4) as wc_pool,
            tc.tile_pool(name="s", bufs=3) as s_pool,
            tc.tile_pool(name="y", bufs=3) as y_pool,
            tc.tile_pool(name="psA", bufs=2, space="PSUM") as psA,
            tc.tile_pool(name="psB", bufs=3, space="PSUM") as psB,
            tc.tile_pool(name="psW", bufs=1, space="PSUM") as psW,
        ):
            # PE p-state warm-up: dependency-free matmuls on a memset tile
            # run during the initial DMA wait so the clock ramps before the
            # first real matmul.
            warm_in = s_pool.tile([P, 512], mybir.dt.bfloat16, tag="warm")
            nc.any.memset(warm_in[:], 0.0)
            warm_ps = psW.tile([P, 512], mybir.dt.float32, tag="warmps")
            for _ in range(8):
                nc.tensor.matmul(
                    warm_ps[:], warm_in[:, 0:P], warm_in[:], start=True, stop=True
                )

            for e in range(EPC):
                C = caps[e]
                chunks = _token_chunks(C)
                WB = 2 * KT * P  # w12 elements per h-block

                # Weights h=0 first (gates the first matmul), then xt chunk 0,
                # then the rest of the weights, then remaining chunks.
                w12t = [None] * HT
                w12t[0] = w12_pool.tile([P, 2, KT, P], B16, tag="w12", name=f"w12_{e}_0")
                nc.sync.dma_start(
                    w12t[0][:],
                    w12_d[e][:, 0:WB].rearrange("p (t k q) -> p t k q", t=2, k=KT),
                )
                xt_t = []
                c0 = 0
                for ci, cs in enumerate(chunks):
                    t = xt_pool.tile([P, KT, cs], B16, tag="xt", name=f"xt_{e}_{ci}")
                    xt_t.append(t)
                    nc.sync.dma_start(
                        t[:],
                        xt_d[e][:, KT * c0 : KT * (c0 + cs)].rearrange(
                            "p (k c) -> p k c", k=KT
                        ),
                    )
                    if ci == 0:
                        for h in range(1, HT):
                            w12t[h] = w12_pool.tile(
                                [P, 2, KT, P], B16, tag="w12", name=f"w12_{e}_{h}"
                            )
                            nc.sync.dma_start(
                                w12t[h][:],
                                w12_d[e][:, h * WB : (h + 1) * WB].rearrange(
                                    "p (t k q) -> p t k q", t=2, k=KT
                                ),
                            )
                    c0 += cs

                ht_sb = ht_pool.tile([P, HT, C], B16, tag="ht")

                # Phase A (chunk-outer): ht = silu(W1.T @ Xt) * (W2.T @ Xt)
                c0 = 0
                for ci, cs in enumerate(chunks):
                    for h in range(HT):
                        ps1 = psA.tile([P, cs], mybir.dt.float32, tag="ps1")
                        ps2 = psA.tile([P, cs], mybir.dt.float32, tag="ps2")
                        for k in range(KT):
                            nc.tensor.matmul(
                                ps1[:],
                                w12t[h][:, 0, k, :],
                                xt_t[ci][:, k, :],
                                start=(k == 0),
                                stop=(k == KT - 1),
                            )
                        for k in range(KT):
                            nc.tensor.matmul(
                                ps2[:],
                                w12t[h][:, 1, k, :],
                                xt_t[ci][:, k, :],
                                start=(k == 0),
                                stop=(k == KT - 1),
                            )
                        s_sb = s_pool.tile([P, 512], mybir.dt.float32, tag="s")
                        nc.scalar.activation(
                            s_sb[:, :cs],
                            ps1[:],
                            mybir.ActivationFunctionType.Silu,
                        )
                        nc.vector.tensor_mul(
                            ht_sb[:, h, c0 : c0 + cs], s_sb[:, :cs], ps2[:]
                        )
                    c0 += cs

                # Phase B: yt = (WC.T @ ht) * gate
                g_sb = g_pool.tile([P, C], F32, tag="g")
                nc.sync.dma_start(g_sb[:], g_d[e][:])
                yt_view = yt_d[e].rearrange("p (d c) -> p d c", d=HT)
                for d in range(HT):
                    wct = wc_pool.tile([P, HT, P], B16, tag="wc")
                    nc.sync.dma_start(
                        wct[:],
                        wc_d[e][:, d * HT * P : (d + 1) * HT * P].rearrange(
                            "p (h q) -> p h q", h=HT
                        ),
                    )
                    c0 = 0
                    for cs in chunks:
                        psy = psB.tile([P, cs], mybir.dt.float32, tag="psy")
                        for h in range(HT):
                            nc.tensor.matmul(
                                psy[:],
                                wct[:, h, :],
                                ht_sb[:, h, c0 : c0 + cs],
                                start=(h == 0),
                                stop=(h == HT - 1),
                            )
                        y_sb = y_pool.tile([P, 512], B16, tag="y")
                        nc.vector.tensor_mul(
                            y_sb[:, :cs], psy[:], g_sb[:, c0 : c0 + cs]
                        )
                        nc.sync.dma_start(
                            yt_view[:, d, c0 : c0 + cs],
                            y_sb[:, :cs],
                        )
                        c0 += cs

    _split_multi_waits(nc)
    return nc


def _pack_w12(wa, wb):
    """[EMB, HID] x2 -> [P, HT*2*KT*P] with layout [p, h, {a,b}, k, q]."""
    ta = wa.astype(BF16).reshape(KT, P, HT, P).transpose(1, 2, 0, 3)  # p h k q
    tb = wb.astype(BF16).reshape(KT, P, HT, P).transpose(1, 2, 0, 3)
    t = np.stack([ta, tb], axis=2)  # p h 2 k q
    return np.ascontiguousarray(t).reshape(P, HT * 2 * KT * P)


def _pack_wc(w):
    """[HID, EMB] -> [P, HT*HT*P] with layout [p, d, h, q]."""
    t = np.ascontiguousarray(
        w.astype(BF16).reshape(HT, P, HT, P).transpose(1, 2, 0, 3)
    )
    return t.reshape(P, HT * HT * P)


def _pack_xt(xe, C, chunks):
    """tokens [n, EMB] -> [P, KT*C] with per-chunk [p, k, cs] blocks."""
    n = xe.shape[0]
    xt = np.zeros((KT, P, C), dtype=BF16)  # k p c
    xt[:, :, :n] = xe.reshape(n, KT, P).transpose(1, 2, 0)
    out = np.empty((P, KT * C), dtype=BF16)
    c0 = 0
    o = 0
    for cs in chunks:
        blk = xt[:, :, c0 : c0 + cs].transpose(1, 0, 2).reshape(P, KT * cs)
        out[:, o : o + KT * cs] = blk
        c0 += cs
        o += KT * cs
    return out


def kernel(x, w_gate, w1, w2, wc):
    trace = bool(int(os.environ.get("BASS_MOE_TRACE", "0")))
    if trace:
        _install_profile_shim()

    import concourse.bass_utils as bass_utils

    bass_utils.upload_artifacts = lambda tmpdir: f"local://{tmpdir}"

    x = np.asarray(x, dtype=np.float32)
    w_gate = np.asarray(w_gate, dtype=np.float32)
    w1 = np.asarray(w1, dtype=np.float32)
    w2 = np.asarray(w2, dtype=np.float32)
    wc = np.asarray(wc, dtype=np.float32)

    b, s, d = x.shape
    xf = x.reshape(-1, d)
    n = xf.shape[0]

    # ---- Router on host (float64: stable ranking + gate values) ----
    logits = xf.astype(np.float64) @ w_gate.astype(np.float64)
    mx = logits.max(axis=1, keepdims=True)
    p = np.exp(logits - mx)
    p /= p.sum(axis=1, keepdims=True)
    top = np.argpartition(-logits, TOPK, axis=1)[:, :TOPK]  # top-2 ids (unordered)

    sel_tok = []  # per expert: token indices
    sel_gate = []  # per expert: gate values
    flat_e = top.ravel()
    flat_t = np.repeat(np.arange(n), TOPK)
    order = np.argsort(flat_e, kind="stable")
    se, st = flat_e[order], flat_t[order]
    bounds = np.searchsorted(se, np.arange(E + 1))
    counts = np.diff(bounds)
    for e in range(E):
        toks = st[bounds[e] : bounds[e + 1]]
        sel_tok.append(toks)
        sel_gate.append(p[toks, e].astype(np.float32))

    # ---- Slot assignment: biggest experts in slot 0, smallest in slot 1,
    # so each slot's uniform capacity hugs its experts' actual counts ----
    rank = np.argsort(-counts, kind="stable")
    slot_experts = [
        [int(rank[core + j * NCORES]) for j in range(EPC)] for core in range(NCORES)
    ]
    caps = []
    for j in range(EPC):
        cmax = max(counts[slot_experts[core][j]] for core in range(NCORES))
        caps.append(max(4, int(-(-cmax // 4) * 4)))

    # ---- Build per-core input maps (expert-parallel: 2 experts/core) ----
    xf_b = xf.astype(BF16)
    in_maps = []
    for core in range(NCORES):
        m = {}
        for j in range(EPC):
            e = slot_experts[core][j]
            C = caps[j]
            chunks = _token_chunks(C)
            toks = sel_tok[e]
            g = np.zeros((C,), dtype=np.float32)
            g[: len(toks)] = sel_gate[e]
            m[f"xt{j}"] = _pack_xt(xf_b[toks], C, chunks)
            m[f"g{j}"] = np.broadcast_to(g, (P, C)).copy()
            m[f"w12_{j}"] = _pack_w12(w1[e], w2[e])
            m[f"wc_{j}"] = _pack_wc(wc[e])
        in_maps.append(m)

    nc = _build_bass(caps)
    res = bass_utils.run_bass_kernel_spmd(
        nc, in_maps, list(range(NCORES)), trace=trace
    )
    if trace:
        kernel.last_exec_time_ns = res.exec_time_ns
        kernel.last_trace = (
            res.instructions_and_trace[1] if res.instructions_and_trace else None
        )

    # ---- Scatter-add back to token order ----
    out = np.zeros((n, d), dtype=np.float32)
    for core in range(NCORES):
        for j in range(EPC):
            e = slot_experts[core][j]
            toks = sel_tok[e]
            C = caps[j]
            yt = (
                np.asarray(res.results[core][f"yt{j}"])
                .reshape(P, HT, C)
                .transpose(1, 0, 2)
                .reshape(EMB, C)
                .astype(np.float32)
            )
            out[toks] += yt[:, : len(toks)].T
    return out.reshape(b, s, d)
